# revision 3
# baseline (speedup 1.0000x reference)
"""Trainium2 Bass kernel for EntityAwareLSTMLayer.

Problem (hardcoded):
  B=1024, T=365, DYN=32, STATIC=27, UNITS=256
  i_gate = sigmoid(x_static @ W_sh + bias_s)            [B, U]   (static, once)
  gx_t   = x_t @ W_ih + bias                            [B, 3U]
  gates  = gx_t + h @ W_hh                              [B, 3U]  (f|o|g)
  c      = sigmoid(f) * c + i_gate * tanh(g)
  h      = sigmoid(o) * tanh(c)
  return h_final                                        [B, U]

Sharding: data-parallel over batch, 8 cores x 128 rows. Batch rows live on
the 128 SBUF partitions; per step the gates are computed by PE matmuls
accumulating K-chunks into PSUM. Weight columns are host-reordered to
[o | f | g] so o and f share one N=512 matmul per K-chunk (one PSUM bank)
and g gets its own N=256 matmul, halving the matmul/LDWEIGHTS count vs
one matmul per gate.

The TensorE clock runs at half speed unless the engine stays busy (~3us
HAM activity window), and the recurrence stalls it every step - so the PE
stream is padded: x-contribution matmuls for future steps are issued into
the gaps, plus junk "warmer" matmuls ordered (same-engine, no semaphores)
right where the PE would otherwise idle waiting on the elementwise chain.

x_dynamic is transposed on-chip via DMA-xbar transposes of [128,128] fp16
chunks (4 timesteps per chunk); timestep t lands at partition group
32*(t%4), so W_ih is replicated at the 4 partition bases.
"""

import numpy as np

B_L = 128  # batch rows per core
T = 365
TP = 368  # T padded to a multiple of 4 for chunked transposes
DYN = 32
STATIC = 27
U = 256
NCORES = 8

LOOKAHEAD = 2  # steps of x-matmul pre-issue (bounded by psum bufs=3)
WARM_A = 3  # filler matmuls after the h-matmul block
WARM_B = 2  # filler matmuls after the transposes

_cached = {}


def _build_program(has_bias: bool):
    from contextlib import ExitStack

    import concourse.bacc as bacc
    import concourse.masks as masks
    import concourse.tile as tile
    from concourse import mybir

    f32 = mybir.dt.float32
    f16 = mybir.dt.float16
    AF = mybir.ActivationFunctionType
    ALU = mybir.AluOpType

    nc = bacc.Bacc("TRN2", target_bir_lowering=False, debug=False)

    # weight_* arrive with columns pre-reordered to [o | f | g] (host side)
    x_dyn = nc.dram_tensor("x_dynamic", [B_L, T * DYN], f32, kind="ExternalInput")
    x_st = nc.dram_tensor("x_static", [B_L, STATIC], f32, kind="ExternalInput")
    w_ih = nc.dram_tensor("weight_ih", [DYN, 3 * U], f32, kind="ExternalInput")
    w_hh = nc.dram_tensor("weight_hh", [U, 3 * U], f32, kind="ExternalInput")
    w_sh = nc.dram_tensor("weight_sh", [STATIC, U], f32, kind="ExternalInput")
    bias = nc.dram_tensor("bias", [1, 3 * U], f32, kind="ExternalInput")
    bias_s = nc.dram_tensor("bias_s", [1, U], f32, kind="ExternalInput")
    out = nc.dram_tensor("out", [B_L, U], f32, kind="ExternalOutput")

    with tile.TileContext(nc) as tc, ExitStack() as ctx:
        const = ctx.enter_context(tc.tile_pool(name="const", bufs=1))
        xtiles = [
            const.tile([128, B_L], f16, tag=f"xt{c}", name=f"xt{c}")
            for c in range(TP // 4)
        ]
        Wih4 = const.tile([128, 3 * U], f16)  # W_ih replicated at 4 bases
        Whh0 = const.tile([128, 3 * U], f16)
        Whh1 = const.tile([128, 3 * U], f16)
        Wshb = const.tile([STATIC + 1, U], f16)  # rows 0-26 W_sh, row 27 bias_s
        xsT = const.tile([128, B_L], f16)
        ident = const.tile([128, 128], f16)
        igate = const.tile([128, U], f16)
        if has_bias:
            ones_row = const.tile([1, B_L], f16)
            bias16 = const.tile([1, 3 * U], f16)

        # [o|f] N=512 psum (one bank) + [g] N=256 psum per step
        psum_fo = ctx.enter_context(tc.tile_pool(name="pfo", bufs=3, space="PSUM"))
        psum_g = ctx.enter_context(tc.tile_pool(name="pg", bufs=3, space="PSUM"))
        psum_t = ctx.enter_context(tc.tile_pool(name="pt", bufs=1, space="PSUM"))

        st = ctx.enter_context(tc.tile_pool(name="state", bufs=2))
        tmp = ctx.enter_context(tc.tile_pool(name="tmp", bufs=3))

        c_prev = st.tile([128, U], f16, tag="c")
        nc.vector.memset(c_prev[:], 0.0)
        hT0 = st.tile([128, B_L], f16, tag="h0")
        nc.vector.memset(hT0[:], 0.0)
        hT1 = st.tile([128, B_L], f16, tag="h1")
        nc.vector.memset(hT1[:], 0.0)

        with tc.tile_pool(name="stage", bufs=1) as stage:
            wst = stage.tile([128, 3 * U], f32)
            nc.sync.dma_start(wst[:], w_hh[0:128, :])
            nc.vector.tensor_copy(Whh0[:], wst[:])
            nc.sync.dma_start(wst[:], w_hh[128:256, :])
            nc.vector.tensor_copy(Whh1[:], wst[:])
            wih32 = stage.tile([DYN, 3 * U], f32)
            nc.sync.dma_start(wih32[:], w_ih[:])
            for g in range(4):
                nc.vector.tensor_copy(Wih4[32 * g : 32 * g + 32, :], wih32[:])
            wsh32 = stage.tile([STATIC, U], f32)
            nc.sync.dma_start(wsh32[:], w_sh[:])
            nc.vector.tensor_copy(Wshb[0:STATIC, :], wsh32[:])
            bs32 = stage.tile([1, U], f32)
            nc.sync.dma_start(bs32[:], bias_s[:])
            bs16 = stage.tile([1, U], f16)
            nc.vector.tensor_copy(bs16[:], bs32[:])
            # partition 27 is not engine-addressable; DMA has no such limit
            nc.sync.dma_start(Wshb[STATIC : STATIC + 1, :], bs16[:])
            if has_bias:
                b32 = stage.tile([1, 3 * U], f32)
                nc.sync.dma_start(b32[:], bias[:])
                nc.vector.tensor_copy(bias16[:], b32[:])
                nc.vector.memset(ones_row[:], 1.0)

            # --- x_static -> transposed [27, 128] + ones row 27 ---
            xst32 = stage.tile([B_L, STATIC], f32)
            nc.sync.dma_start(xst32[:], x_st[:])
            xst16 = stage.tile([B_L, 128], f16)
            nc.vector.memset(xst16[:], 0.0)
            nc.vector.tensor_copy(xst16[:, 0:STATIC], xst32[:])
            nc.vector.memset(xst16[:, STATIC : STATIC + 1], 1.0)
            nc.sync.dma_start_transpose(xsT[:], xst16[:])

            masks.make_identity(nc, ident[:])

            # --- i_gate = sigmoid(x_static @ W_sh + bias_s) ---
            ig_ps = psum_g.tile([128, U], f32, tag="ps_g")
            nc.tensor.matmul(
                ig_ps[:], xsT[0 : STATIC + 1, :], Wshb[:], start=True, stop=True
            )
            nc.scalar.activation(igate[:], ig_ps[:], AF.Sigmoid)

            # --- x_dynamic: load fp32, convert fp16, transpose in chunks ---
            x16 = stage.tile([B_L, TP * DYN], f16)
            nc.vector.memset(x16[:, T * DYN :], 0.0)
            NCH = 4
            split = NCH * 128
            xs32a = stage.tile([B_L, split], f32)
            nc.sync.dma_start(xs32a[:], x_dyn[:, 0:split])
            nc.vector.tensor_copy(x16[:, 0:split], xs32a[:])
            for c in range(NCH):
                nc.sync.dma_start_transpose(
                    xtiles[c][:], x16[:, c * 128 : (c + 1) * 128]
                )
            xs32b = stage.tile([B_L, T * DYN - split], f32)
            nc.sync.dma_start(xs32b[:], x_dyn[:, split:])
            nc.vector.tensor_copy(x16[:, split : T * DYN], xs32b[:])
            for c in range(NCH, TP // 4):
                nc.sync.dma_start_transpose(
                    xtiles[c][:], x16[:, c * 128 : (c + 1) * 128]
                )

        def x_matmuls(t, ps_fo, ps_g):
            g4 = 32 * (t % 4)
            cc = t // 4
            xt = xtiles[cc][g4 : g4 + 32, :]
            wx = Wih4[g4 : g4 + 32, :]
            mms = []
            mms.append(
                nc.tensor.matmul(
                    ps_fo[:],
                    xt,
                    wx[:, 0 : 2 * U],
                    start=True,
                    stop=False,
                    tile_position=(g4, 0),
                )
            )
            mms.append(
                nc.tensor.matmul(
                    ps_g[:],
                    xt,
                    wx[:, 2 * U : 3 * U],
                    start=True,
                    stop=False,
                    tile_position=(g4, 0),
                )
            )
            if has_bias:
                mms.append(
                    nc.tensor.matmul(
                        ps_fo[:],
                        ones_row[:],
                        bias16[:, 0 : 2 * U],
                        start=False,
                        stop=False,
                    )
                )
                mms.append(
                    nc.tensor.matmul(
                        ps_g[:],
                        ones_row[:],
                        bias16[:, 2 * U : 3 * U],
                        start=False,
                        stop=False,
                    )
                )
            return mms

        def new_ps():
            ps_fo = psum_fo.tile([128, 2 * U], f32, tag="ps_fo")
            ps_g = psum_g.tile([128, U], f32, tag="ps_g")
            return (ps_fo, ps_g)

        # junk psum for PE warm-keeper matmuls (never read)
        junkp = ctx.enter_context(tc.tile_pool(name="junk", bufs=1, space="PSUM"))
        junk = junkp.tile([128, 2 * U], f32, tag="junk")

        def warm_fill(n, after):
            prev = after
            for _ in range(n):
                f = nc.tensor.matmul(
                    junk[:], ident[:], Whh0[:, 0 : 2 * U], start=True, stop=True
                )
                tile.add_dep_helper(f.ins, prev.ins, sync=False, reason="warm order")
                prev = f
            return prev

        # x contributions pre-issued LOOKAHEAD+1 steps ahead of the h-matmuls
        pss = []
        for j in range(LOOKAHEAD + 1):
            ps = new_ps()
            x_matmuls(j, *ps)
            pss.append(ps)

        for t in range(T):
            last = t == T - 1
            ps_fo, ps_g = pss[0]
            pss = pss[1:]
            # h-matmuls: [o|f] N=512 then [g] N=256 per K-chunk; explicit
            # order so sigmoid(f,o) can start after the two fo matmuls
            mms = []
            mms.append(
                nc.tensor.matmul(
                    ps_fo[:], hT0[:], Whh0[:, 0 : 2 * U], start=False, stop=False
                )
            )
            mms.append(
                nc.tensor.matmul(
                    ps_fo[:], hT1[:], Whh1[:, 0 : 2 * U], start=False, stop=True
                )
            )
            mms.append(
                nc.tensor.matmul(
                    ps_g[:], hT0[:], Whh0[:, 2 * U : 3 * U], start=False, stop=False
                )
            )
            mms.append(
                nc.tensor.matmul(
                    ps_g[:], hT1[:], Whh1[:, 2 * U : 3 * U], start=False, stop=True
                )
            )
            for a, b in zip(mms[1:], mms[:-1]):
                tile.add_dep_helper(a.ins, b.ins, sync=False, reason="mm order")

            # pre-issue x matmuls for step t+LOOKAHEAD+1 right after the h
            # matmuls: independent work that keeps the PE stream dense
            tx = t + LOOKAHEAD + 1
            if tx < T:
                ps_n = new_ps()
                xmm = x_matmuls(tx, *ps_n)
                tile.add_dep_helper(
                    xmm[0].ins, mms[-1].ins, sync=False, reason="x after h"
                )
                for a, b in zip(xmm[1:], xmm[:-1]):
                    tile.add_dep_helper(a.ins, b.ins, sync=False, reason="x order")
                pss.append(ps_n)
                fill_anchor = xmm[-1]
            else:
                fill_anchor = mms[-1]
            if not last:
                warm_fill(WARM_A, fill_anchor)

            # sfo = [sigmoid(o) | sigmoid(f)] in one FD=512 op
            sfo = tmp.tile([128, 2 * U], f16, tag="sfo")
            nc.scalar.activation(sfo[:], ps_fo[:], AF.Sigmoid)
            tg = tmp.tile([128, U], f16, tag="tg")
            nc.scalar.activation(tg[:], ps_g[:], AF.Tanh)

            m1 = tmp.tile([128, U], f16, tag="m1")
            nc.vector.tensor_mul(m1[:], sfo[:, U : 2 * U], c_prev[:])
            m2 = tmp.tile([128, U], f16, tag="m2")
            nc.vector.tensor_mul(m2[:], igate[:], tg[:])
            c_new = st.tile([128, U], f16, tag="c")
            nc.vector.tensor_add(c_new[:], m1[:], m2[:])

            if last:
                tch = tmp.tile([128, U], f32, tag="tc32")
                nc.scalar.activation(tch[:], c_new[:], AF.Tanh)
                h_out = tmp.tile([128, U], f32, tag="hout")
                nc.vector.tensor_mul(h_out[:], sfo[:, 0:U], tch[:])
                nc.sync.dma_start(out[:], h_out[:])
            else:
                # tail split into u-halves so transpose/copy/h-matmul of half 0
                # start while half 1 is still in ACT/DVE
                hTn = [None, None]
                trs = []
                for half in (0, 1):
                    lo, hi = 128 * half, 128 * (half + 1)
                    tch = tmp.tile([128, 128], f16, tag=f"tc{half}")
                    nc.scalar.activation(tch[:], c_new[:, lo:hi], AF.Tanh)
                    hh = tmp.tile([128, 128], f16, tag=f"hh{half}")
                    nc.vector.tensor_mul(hh[:], sfo[:, lo:hi], tch[:])
                    pp = psum_t.tile([128, 128], f16, tag="pt")
                    trs.append(nc.tensor.transpose(pp[:], hh[:], ident[:]))
                    ht_new = st.tile([128, B_L], f16, tag=f"h{half}")
                    nc.vector.tensor_copy(ht_new[:], pp[:])
                    hTn[half] = ht_new
                hT0, hT1 = hTn
                warm_fill(WARM_B, trs[1])
            c_prev = c_new

    nc.compile()
    return nc


def get_program(has_bias: bool = False):
    if has_bias not in _cached:
        _cached[has_bias] = _build_program(has_bias)
    return _cached[has_bias]


def _reorder_cols(w):
    # [f | o | g] -> [o | f | g]
    return np.concatenate([w[:, U : 2 * U], w[:, 0:U], w[:, 2 * U : 3 * U]], axis=1)


def make_in_maps(inputs):
    x_dynamic = np.asarray(inputs["x_dynamic"], dtype=np.float32)
    x_static = np.asarray(inputs["x_static"], dtype=np.float32)
    w_ih = np.ascontiguousarray(
        _reorder_cols(np.asarray(inputs["weight_ih"], dtype=np.float32))
    )
    w_hh = np.ascontiguousarray(
        _reorder_cols(np.asarray(inputs["weight_hh"], dtype=np.float32))
    )
    w_sh = np.ascontiguousarray(np.asarray(inputs["weight_sh"], dtype=np.float32))
    bias = np.ascontiguousarray(
        _reorder_cols(np.asarray(inputs["bias"], dtype=np.float32).reshape(1, 3 * U))
    )
    bias_s = np.ascontiguousarray(
        np.asarray(inputs["bias_s"], dtype=np.float32).reshape(1, U)
    )
    in_maps = []
    for i in range(NCORES):
        sl = slice(i * B_L, (i + 1) * B_L)
        in_maps.append(
            {
                "x_dynamic": np.ascontiguousarray(
                    x_dynamic[sl].reshape(B_L, T * DYN)
                ),
                "x_static": np.ascontiguousarray(x_static[sl]),
                "weight_ih": w_ih,
                "weight_hh": w_hh,
                "weight_sh": w_sh,
                "bias": bias,
                "bias_s": bias_s,
            }
        )
    return in_maps


def kernel(**inputs) -> np.ndarray:
    from concourse.bass_utils import run_bass_kernel_spmd

    has_bias = bool(np.any(np.asarray(inputs["bias"])))
    nc = get_program(has_bias)
    in_maps = make_in_maps(inputs)
    res = run_bass_kernel_spmd(nc, in_maps, core_ids=list(range(NCORES)))
    return np.concatenate([r["out"] for r in res.results], axis=0).astype(np.float32)


# revision 4
# speedup vs baseline: 1.3020x; 1.3020x over previous
"""Trainium2 Bass kernel for EntityAwareLSTMLayer.

Problem (hardcoded):
  B=1024, T=365, DYN=32, STATIC=27, UNITS=256
  i_gate = sigmoid(x_static @ W_sh + bias_s)            [B, U]   (static, once)
  gx_t   = x_t @ W_ih + bias                            [B, 3U]
  gates  = gx_t + h @ W_hh                              [B, 3U]  (f|o|g)
  c      = sigmoid(f) * c + i_gate * tanh(g)
  h      = sigmoid(o) * tanh(c)
  return h_final                                        [B, U]

Sharding: data-parallel over batch, 8 cores x 128 rows. Batch rows live on
the 128 SBUF partitions; per step the gates are computed by PE matmuls
accumulating K-chunks into PSUM. Weight columns are host-reordered to
[o | f | g] so o and f share one N=512 matmul per K-chunk (one PSUM bank)
and g gets its own N=256 matmul, halving the matmul/LDWEIGHTS count vs
one matmul per gate.

The TensorE clock runs at half speed unless the engine stays busy (~3us
HAM activity window), and the recurrence stalls it every step - so the PE
stream is padded: x-contribution matmuls for future steps are issued into
the gaps, plus junk "warmer" matmuls ordered (same-engine, no semaphores)
right where the PE would otherwise idle waiting on the elementwise chain.

x_dynamic is transposed on-chip via DMA-xbar transposes of [128,128] fp16
chunks (4 timesteps per chunk); timestep t lands at partition group
32*(t%4), so W_ih is replicated at the 4 partition bases.
"""

import numpy as np

B_L = 128  # batch rows per core
T = 365
TP = 368  # T padded to a multiple of 4 for chunked transposes
DYN = 32
STATIC = 27
U = 256
NCORES = 8

WARM_A = 10  # N=128 filler matmuls after the x-matmul block

_cached = {}


def _build_program(has_bias: bool):
    from contextlib import ExitStack

    import concourse.bacc as bacc
    import concourse.masks as masks
    import concourse.tile as tile
    from concourse import mybir

    f32 = mybir.dt.float32
    f16 = mybir.dt.float16
    AF = mybir.ActivationFunctionType
    ALU = mybir.AluOpType

    nc = bacc.Bacc("TRN2", target_bir_lowering=False, debug=False)

    # weight_* arrive with columns pre-reordered to [o | f | g] (host side)
    x_dyn = nc.dram_tensor("x_dynamic", [B_L, T * DYN], f32, kind="ExternalInput")
    x_st = nc.dram_tensor("x_static", [B_L, STATIC], f32, kind="ExternalInput")
    w_ih = nc.dram_tensor("weight_ih", [DYN, 3 * U], f32, kind="ExternalInput")
    w_hh = nc.dram_tensor("weight_hh", [U, 3 * U], f32, kind="ExternalInput")
    w_sh = nc.dram_tensor("weight_sh", [STATIC, U], f32, kind="ExternalInput")
    bias = nc.dram_tensor("bias", [1, 3 * U], f32, kind="ExternalInput")
    bias_s = nc.dram_tensor("bias_s", [1, U], f32, kind="ExternalInput")
    out = nc.dram_tensor("out", [B_L, U], f32, kind="ExternalOutput")

    with tile.TileContext(nc) as tc, ExitStack() as ctx:
        const = ctx.enter_context(tc.tile_pool(name="const", bufs=1))
        xtiles = [
            const.tile([128, B_L], f16, tag=f"xt{c}", name=f"xt{c}")
            for c in range(TP // 4)
        ]
        Wih4 = const.tile([128, 3 * U], f16)  # W_ih replicated at 4 bases
        Whh0 = const.tile([128, 3 * U], f16)
        Whh1 = const.tile([128, 3 * U], f16)
        Wshb = const.tile([STATIC + 1, U], f16)  # rows 0-26 W_sh, row 27 bias_s
        xsT = const.tile([128, B_L], f16)
        ident = const.tile([128, 128], f16)
        igate = const.tile([128, U], f16)
        if has_bias:
            ones_row = const.tile([1, B_L], f16)
            bias16 = const.tile([1, 3 * U], f16)

        # [o|f] N=512 psum (one bank) + [g] N=256 psum per step
        psum_fo = ctx.enter_context(tc.tile_pool(name="pfo", bufs=3, space="PSUM"))
        psum_g = ctx.enter_context(tc.tile_pool(name="pg", bufs=2, space="PSUM"))
        psum_t = ctx.enter_context(tc.tile_pool(name="pt", bufs=2, space="PSUM"))

        st = ctx.enter_context(tc.tile_pool(name="state", bufs=2))
        tmp = ctx.enter_context(tc.tile_pool(name="tmp", bufs=3))

        c_prev = st.tile([128, U], f16, tag="c")
        nc.vector.memset(c_prev[:], 0.0)
        hT0 = st.tile([128, B_L], f16, tag="h0")
        nc.vector.memset(hT0[:], 0.0)
        hT1 = st.tile([128, B_L], f16, tag="h1")
        nc.vector.memset(hT1[:], 0.0)

        with tc.tile_pool(name="stage", bufs=1) as stage:
            wst = stage.tile([128, 3 * U], f32)
            nc.sync.dma_start(wst[:], w_hh[0:128, :])
            nc.vector.tensor_copy(Whh0[:], wst[:])
            nc.sync.dma_start(wst[:], w_hh[128:256, :])
            nc.vector.tensor_copy(Whh1[:], wst[:])
            wih32 = stage.tile([DYN, 3 * U], f32)
            nc.sync.dma_start(wih32[:], w_ih[:])
            for g in range(4):
                nc.vector.tensor_copy(Wih4[32 * g : 32 * g + 32, :], wih32[:])
            wsh32 = stage.tile([STATIC, U], f32)
            nc.sync.dma_start(wsh32[:], w_sh[:])
            nc.vector.tensor_copy(Wshb[0:STATIC, :], wsh32[:])
            bs32 = stage.tile([1, U], f32)
            nc.sync.dma_start(bs32[:], bias_s[:])
            bs16 = stage.tile([1, U], f16)
            nc.vector.tensor_copy(bs16[:], bs32[:])
            # partition 27 is not engine-addressable; DMA has no such limit
            nc.sync.dma_start(Wshb[STATIC : STATIC + 1, :], bs16[:])
            if has_bias:
                b32 = stage.tile([1, 3 * U], f32)
                nc.sync.dma_start(b32[:], bias[:])
                nc.vector.tensor_copy(bias16[:], b32[:])
                nc.vector.memset(ones_row[:], 1.0)

            # --- x_static -> transposed [27, 128] + ones row 27 ---
            xst32 = stage.tile([B_L, STATIC], f32)
            nc.sync.dma_start(xst32[:], x_st[:])
            xst16 = stage.tile([B_L, 128], f16)
            nc.vector.memset(xst16[:], 0.0)
            nc.vector.tensor_copy(xst16[:, 0:STATIC], xst32[:])
            nc.vector.memset(xst16[:, STATIC : STATIC + 1], 1.0)
            nc.sync.dma_start_transpose(xsT[:], xst16[:])

            masks.make_identity(nc, ident[:])

            # --- i_gate = sigmoid(x_static @ W_sh + bias_s) ---
            ig_ps = psum_g.tile([128, U], f32, tag="ps_g")
            nc.tensor.matmul(
                ig_ps[:], xsT[0 : STATIC + 1, :], Wshb[:], start=True, stop=True
            )
            nc.scalar.activation(igate[:], ig_ps[:], AF.Sigmoid)

            # --- x_dynamic: load fp32, convert fp16, transpose in chunks ---
            x16 = stage.tile([B_L, TP * DYN], f16)
            nc.vector.memset(x16[:, T * DYN :], 0.0)
            NCH = 4
            split = NCH * 128
            xs32a = stage.tile([B_L, split], f32)
            nc.sync.dma_start(xs32a[:], x_dyn[:, 0:split])
            nc.vector.tensor_copy(x16[:, 0:split], xs32a[:])
            for c in range(NCH):
                nc.sync.dma_start_transpose(
                    xtiles[c][:], x16[:, c * 128 : (c + 1) * 128]
                )
            xs32b = stage.tile([B_L, T * DYN - split], f32)
            nc.sync.dma_start(xs32b[:], x_dyn[:, split:])
            nc.vector.tensor_copy(x16[:, split : T * DYN], xs32b[:])
            for c in range(NCH, TP // 4):
                nc.sync.dma_start_transpose(
                    xtiles[c][:], x16[:, c * 128 : (c + 1) * 128]
                )

        def x_fo(t, ps_fo):
            g4 = 32 * (t % 4)
            cc = t // 4
            xt = xtiles[cc][g4 : g4 + 32, :]
            wx = Wih4[g4 : g4 + 32, :]
            mms = [
                nc.tensor.matmul(
                    ps_fo[:],
                    xt,
                    wx[:, 0 : 2 * U],
                    start=True,
                    stop=False,
                    tile_position=(g4, 0),
                )
            ]
            if has_bias:
                mms.append(
                    nc.tensor.matmul(
                        ps_fo[:],
                        ones_row[:],
                        bias16[:, 0 : 2 * U],
                        start=False,
                        stop=False,
                    )
                )
            return mms

        def x_g(t, ps_g):
            g4 = 32 * (t % 4)
            cc = t // 4
            xt = xtiles[cc][g4 : g4 + 32, :]
            wx = Wih4[g4 : g4 + 32, :]
            mms = [
                nc.tensor.matmul(
                    ps_g[:],
                    xt,
                    wx[:, 2 * U : 3 * U],
                    start=True,
                    stop=False,
                    tile_position=(g4, 0),
                )
            ]
            if has_bias:
                mms.append(
                    nc.tensor.matmul(
                        ps_g[:],
                        ones_row[:],
                        bias16[:, 2 * U : 3 * U],
                        start=False,
                        stop=False,
                    )
                )
            return mms

        # junk psum for PE warm-keeper matmuls (never read)
        junkp = ctx.enter_context(tc.tile_pool(name="junk", bufs=1, space="PSUM"))
        junk = junkp.tile([128, 128], f32, tag="junk")

        def warm_fill(n, after):
            prev = after
            for _ in range(n):
                f = nc.tensor.matmul(
                    junk[:], ident[:], Whh0[:, 0:128], start=True, stop=True
                )
                tile.add_dep_helper(f.ins, prev.ins, sync=False, reason="warm order")
                prev = f
            return prev

        # x contributions pre-issued ahead of the h-matmuls: fo 2 steps, g 1
        fo_ps = []
        for j in range(2):
            ps = psum_fo.tile([128, 2 * U], f32, tag="ps_fo")
            x_fo(j, ps)
            fo_ps.append(ps)
        g_ps = []
        ps = psum_g.tile([128, U], f32, tag="ps_g")
        x_g(0, ps)
        g_ps.append(ps)

        for t in range(T):
            last = t == T - 1
            ps_fo = fo_ps.pop(0)
            ps_g = g_ps.pop(0)
            # h-matmuls: [o|f] N=512 then [g] N=256 per K-chunk; explicit
            # order so sigmoid(f,o) can start after the two fo matmuls
            mms = []
            mms.append(
                nc.tensor.matmul(
                    ps_fo[:], hT0[:], Whh0[:, 0 : 2 * U], start=False, stop=False
                )
            )
            mms.append(
                nc.tensor.matmul(
                    ps_fo[:], hT1[:], Whh1[:, 0 : 2 * U], start=False, stop=True
                )
            )
            mms.append(
                nc.tensor.matmul(
                    ps_g[:], hT0[:], Whh0[:, 2 * U : 3 * U], start=False, stop=False
                )
            )
            mms.append(
                nc.tensor.matmul(
                    ps_g[:], hT1[:], Whh1[:, 2 * U : 3 * U], start=False, stop=True
                )
            )
            for a, b in zip(mms[1:], mms[:-1]):
                tile.add_dep_helper(a.ins, b.ins, sync=False, reason="mm order")

            # pre-issue x matmuls (fo: t+2, g: t+1) right after the h
            # matmuls: independent work that keeps the PE stream dense
            anchor = mms[-1]
            if t + 2 < T:
                ps_n = psum_fo.tile([128, 2 * U], f32, tag="ps_fo")
                xmm = x_fo(t + 2, ps_n)
                tile.add_dep_helper(
                    xmm[0].ins, anchor.ins, sync=False, reason="x after h"
                )
                fo_ps.append(ps_n)
                anchor = xmm[-1]
            if t + 1 < T:
                ps_n = psum_g.tile([128, U], f32, tag="ps_g")
                xmm = x_g(t + 1, ps_n)
                tile.add_dep_helper(
                    xmm[0].ins, anchor.ins, sync=False, reason="xg after xfo"
                )
                g_ps.append(ps_n)
                anchor = xmm[-1]
            if not last:
                warm_fill(WARM_A, anchor)

            # sigmoid(f) first (gates the c chain), sigmoid(o) late: it is
            # only needed for the h products ~1.5us later
            sfo = tmp.tile([128, 2 * U], f16, tag="sfo")
            af = nc.scalar.activation(sfo[:, U : 2 * U], ps_fo[:, U : 2 * U], AF.Sigmoid)
            tg = tmp.tile([128, U], f16, tag="tg")
            ag = nc.scalar.activation(tg[:], ps_g[:], AF.Tanh)
            ao = nc.scalar.activation(sfo[:, 0:U], ps_fo[:, 0:U], AF.Sigmoid)
            tile.add_dep_helper(ag.ins, af.ins, sync=False, reason="act order")
            tile.add_dep_helper(ao.ins, ag.ins, sync=False, reason="act order")

            m1 = tmp.tile([128, U], f16, tag="m1")
            nc.vector.tensor_mul(m1[:], sfo[:, U : 2 * U], c_prev[:])
            m2 = tmp.tile([128, U], f16, tag="m2")
            nc.vector.tensor_mul(m2[:], igate[:], tg[:])
            c_new = st.tile([128, U], f16, tag="c")
            nc.vector.tensor_add(c_new[:], m1[:], m2[:])

            if last:
                tch = tmp.tile([128, U], f32, tag="tc32")
                nc.scalar.activation(tch[:], c_new[:], AF.Tanh)
                h_out = tmp.tile([128, U], f32, tag="hout")
                nc.vector.tensor_mul(h_out[:], sfo[:, 0:U], tch[:])
                nc.sync.dma_start(out[:], h_out[:])
            else:
                # tail split into u-halves so transpose/copy/h-matmul of half 0
                # start while half 1 is still in ACT/DVE
                hTn = [None, None]
                trs = []
                for half in (0, 1):
                    lo, hi = 128 * half, 128 * (half + 1)
                    tch = tmp.tile([128, 128], f16, tag=f"tc{half}")
                    nc.scalar.activation(tch[:], c_new[:, lo:hi], AF.Tanh)
                    hh = tmp.tile([128, 128], f16, tag=f"hh{half}")
                    nc.vector.tensor_mul(hh[:], sfo[:, lo:hi], tch[:])
                    pp = psum_t.tile([128, 128], f16, tag="pt")
                    trs.append(nc.tensor.transpose(pp[:], hh[:], ident[:]))
                    ht_new = st.tile([128, B_L], f16, tag=f"h{half}")
                    nc.vector.tensor_copy(ht_new[:], pp[:])
                    hTn[half] = ht_new
                hT0, hT1 = hTn
            c_prev = c_new

    nc.compile()
    return nc


def get_program(has_bias: bool = False):
    if has_bias not in _cached:
        _cached[has_bias] = _build_program(has_bias)
    return _cached[has_bias]


def _reorder_cols(w):
    # [f | o | g] -> [o | f | g]
    return np.concatenate([w[:, U : 2 * U], w[:, 0:U], w[:, 2 * U : 3 * U]], axis=1)


def make_in_maps(inputs):
    x_dynamic = np.asarray(inputs["x_dynamic"], dtype=np.float32)
    x_static = np.asarray(inputs["x_static"], dtype=np.float32)
    w_ih = np.ascontiguousarray(
        _reorder_cols(np.asarray(inputs["weight_ih"], dtype=np.float32))
    )
    w_hh = np.ascontiguousarray(
        _reorder_cols(np.asarray(inputs["weight_hh"], dtype=np.float32))
    )
    w_sh = np.ascontiguousarray(np.asarray(inputs["weight_sh"], dtype=np.float32))
    bias = np.ascontiguousarray(
        _reorder_cols(np.asarray(inputs["bias"], dtype=np.float32).reshape(1, 3 * U))
    )
    bias_s = np.ascontiguousarray(
        np.asarray(inputs["bias_s"], dtype=np.float32).reshape(1, U)
    )
    in_maps = []
    for i in range(NCORES):
        sl = slice(i * B_L, (i + 1) * B_L)
        in_maps.append(
            {
                "x_dynamic": np.ascontiguousarray(
                    x_dynamic[sl].reshape(B_L, T * DYN)
                ),
                "x_static": np.ascontiguousarray(x_static[sl]),
                "weight_ih": w_ih,
                "weight_hh": w_hh,
                "weight_sh": w_sh,
                "bias": bias,
                "bias_s": bias_s,
            }
        )
    return in_maps


def kernel(**inputs) -> np.ndarray:
    from concourse.bass_utils import run_bass_kernel_spmd

    has_bias = bool(np.any(np.asarray(inputs["bias"])))
    nc = get_program(has_bias)
    in_maps = make_in_maps(inputs)
    res = run_bass_kernel_spmd(nc, in_maps, core_ids=list(range(NCORES)))
    return np.concatenate([r["out"] for r in res.results], axis=0).astype(np.float32)
